# revision 4
# baseline (speedup 1.0000x reference)
"""Trainium2 Bass kernel for the 2-layer GCN encoder + global mean pool.

Design (v7):
  - Nodes sharded contiguously across 8 cores (12500/core, 98 blocks of 128).
  - Per-layer halo exchange: AllGather of the dis-scaled transformed features,
    split into NW=4 chunks of 3125 rows; each chunk is issued as soon as the
    blocks covering its rows are stored (interleaved into the front-end /
    layer-1 epilogue), so collectives overlap aggregation. Window size
    25000 < 32768 keeps gather indices within int16 range.
  - Edge messages fetched with GPSIMD dma_gather (1024 idx/instr, 4 SWDGE
    queues round-robin) from the gathered table; ~35ns/descriptor/SDMA-engine
    for random 256B rows is the dominant cost (~0.9ms of the ~1.07ms total).
    The last sub-gather of each (group,window) passes the exact edge count
    (16-rounded) so ceil-to-128 tail pads generate no descriptors; the gather
    tile ring is memset once at start so untouched tails stay finite.
  - Edges grouped by (dst block, src window), packed edge-granularly; a column
    straddling two adjacent blocks serves both via parity-encoded slots (odd
    blocks +128 matched against an iota+128 pattern).
  - One-hot scatter matrices built on DVE in batched form: ONE
    tensor_tensor(is_equal) per (block, window) span, using slots duplicated
    x2 in bf16 with access pattern [(2,ncols),(0,64),(1,2)] against a tiled
    iota constant -- keeps the 2x_1p DVE perf mode while amortizing
    instruction overhead ~5x vs per-column tensor_scalar.
  - PE matmuls accumulate messages per dst block (layer 2 in transposed
    orientation); z2 stays in SBUF; mean pool consumes it directly.
"""

import math
import os

import ml_dtypes
import numpy as np

P = 128
N_NODES = 100000
N_EDGES = 1600000
NUM_GRAPHS = 1000
IN_DIM, HID_DIM, OUT_DIM = 256, 128, 64
N_CORES = 8
NW = 4            # AllGather chunks == gather windows
AGG_G = 10        # dst blocks per gather mega-group

BF16 = ml_dtypes.bfloat16
PAD_SLOT = 300.0


class Plan:
    pass


def make_plan(x, W1, b1, W2, b2, edge_index, batch,
              n_nodes=N_NODES, num_graphs=NUM_GRAPHS, n_cores=N_CORES):
    pl = Plan()
    n_pc = n_nodes // n_cores
    assert n_pc * n_cores == n_nodes
    n_blk = math.ceil(n_pc / P)
    n_pad = n_blk * P
    Q = n_pc // NW
    assert Q * NW == n_pc
    Wrows = n_cores * Q
    assert Wrows < 32768
    pl.n_nodes, pl.num_graphs, pl.n_cores = n_nodes, num_graphs, n_cores
    pl.n_pc, pl.n_blk, pl.n_pad = n_pc, n_blk, n_pad
    pl.Q, pl.Wrows = Q, Wrows
    pl.d_in, pl.d_hid, pl.d_out = x.shape[1], W1.shape[1], W2.shape[1]

    src = np.asarray(edge_index[0], dtype=np.int64)
    dst = np.asarray(edge_index[1], dtype=np.int64)
    batch = np.asarray(batch, dtype=np.int64)

    deg = np.bincount(dst, minlength=n_nodes).astype(np.float64) + 1.0
    dis = (1.0 / np.sqrt(deg)).astype(np.float32)

    core = dst // n_pc
    loc = dst - core * n_pc
    blk = loc // P
    slot = loc % P

    ks = src // n_pc
    r0 = src - ks * n_pc
    w = r0 // Q
    rel = ks * Q + (r0 % Q)          # row within window w of y_full

    # groups of blocks
    groups = []
    b0 = 0
    while b0 < n_blk:
        groups.append((b0, min(b0 + AGG_G, n_blk)))
        b0 += AGG_G
    pl.groups = groups
    n_g = len(groups)

    # per-core edge sort by (blk, w)
    key = core * (n_blk * NW) + blk * NW + w
    order = np.argsort(key, kind="stable")
    counts = np.bincount(key, minlength=n_cores * n_blk * NW) \
        .reshape(n_cores, n_blk, NW)
    cols_bw = np.ceil(counts.max(axis=0) / P).astype(np.int64)   # [n_blk, NW]

    # edge-granular packing: block b's segment in (m,w) is [t_bw, t_bw+s_bw)
    # positions (s = max count over cores); only the instruction total is
    # rounded to 128.  Straddle columns serve two adjacent blocks via
    # parity-encoded slots (odd blocks' slots stored +128).
    s_bw = counts.max(axis=0).astype(np.int64)               # [n_blk, NW]
    seg_t = np.zeros((n_blk, NW), dtype=np.int64)            # local pos
    col_w_start = np.zeros((n_g, NW), dtype=np.int64)        # global col
    inst_cols = np.zeros((n_g, NW), dtype=np.int64)
    inst_tot = np.zeros((n_g, NW), dtype=np.int64)
    inst_idx_off = np.zeros((n_g, NW), dtype=np.int64)
    c = 0
    ioff = 0
    for m, (lo, hi) in enumerate(groups):
        for wdw in range(NW):
            col_w_start[m, wdw] = c
            inst_idx_off[m, wdw] = ioff
            t = 0
            for b in range(lo, hi):
                seg_t[b, wdw] = t
                t += int(s_bw[b, wdw])
            nc_cols = (t + P - 1) // P
            inst_cols[m, wdw] = nc_cols
            inst_tot[m, wdw] = t
            c += nc_cols
            ioff += nc_cols * 8
            assert nc_cols * P <= 15872
    C_tot = int(c)
    pl.s_bw, pl.seg_t = s_bw, seg_t
    pl.col_w_start, pl.inst_cols, pl.inst_idx_off = \
        col_w_start, inst_cols, inst_idx_off
    pl.inst_tot = inst_tot
    pl.C_tot = C_tot
    pl.n_groups = n_g

    # fill per-core slot/idx arrays
    sorted_key = key[order]
    block_start = np.concatenate([[0], np.cumsum(counts.reshape(-1))])[:-1]
    rank = np.arange(len(order)) - block_start[sorted_key]
    core_o = core[order]
    blk_o = blk[order]
    w_o = w[order]
    slot_o = slot[order]
    rel_o = rel[order]

    grp_of_blk = np.zeros(n_blk, dtype=np.int64)
    for m, (lo, hi) in enumerate(groups):
        grp_of_blk[lo:hi] = m
    m_o = grp_of_blk[blk_o]
    pos = seg_t[blk_o, w_o] + rank           # instruction-local position
    colpos = col_w_start[m_o, w_o] + pos // P           # global column
    ppos = pos % P                                      # partition in column

    slots_all = np.full((n_cores, P, C_tot), PAD_SLOT, dtype=np.float32)
    slots_all[core_o, ppos, colpos] = \
        (slot_o + 128 * (blk_o % 2)).astype(np.float32)
    pl.slots_all = slots_all

    # batched one-hot build operands: slots duplicated x2 along columns (for
    # the 2x_1p DVE mode trick), bf16 (all values <= 511 exact)
    pl.slots2 = np.repeat(slots_all, 2, axis=2).astype(BF16)

    # per-(b,w) column spans and the max span width
    span_lo = np.zeros((n_blk, NW), dtype=np.int64)
    span_n = np.zeros((n_blk, NW), dtype=np.int64)
    for b in range(n_blk):
        for wdw in range(NW):
            s = int(s_bw[b, wdw])
            if s == 0:
                continue
            t = int(seg_t[b, wdw])
            lc0 = t // P
            lc1 = (t + s - 1) // P
            span_lo[b, wdw] = lc0
            span_n[b, wdw] = lc1 - lc0 + 1
    pl.span_lo, pl.span_n = span_lo, span_n
    pl.max_span = int(span_n.max())

    # tiled iota constants for the batched is_equal (parity 0 and +128)
    ms = pl.max_span
    io_t = np.tile(np.arange(P, dtype=np.float32), (P, ms))
    pl.iorep0 = io_t.astype(BF16).copy()
    pl.iorep1 = (io_t + 128.0).astype(BF16).copy()

    idxcol = inst_idx_off[m_o, w_o] + pos // 16
    idxrow = pos % 16
    idx16 = np.zeros((n_cores, 16, C_tot * 8), dtype=np.int16)
    idx16[core_o, idxrow, idxcol] = rel_o.astype(np.int16)
    pl.idx16_all = np.tile(idx16, (1, 8, 1))            # replicate to 128 rows

    # per-node scalars laid out [core][P, n_blk]
    def node_layout(vals, pad=0.0):
        out = np.full((n_cores, P, n_blk), pad, dtype=np.float32)
        v = vals.reshape(n_cores, n_pc)
        for k in range(n_cores):
            full = np.full(n_pad, pad, dtype=np.float32)
            full[:n_pc] = v[k]
            out[k] = full.reshape(n_blk, P).T
        return out

    pl.dis_t = node_layout(dis)
    cnt = np.bincount(batch, minlength=num_graphs).astype(np.float64)
    recip_g = (1.0 / np.maximum(cnt, 1.0)).astype(np.float32)
    pl.recip_t = node_layout(recip_g[batch])

    # pooling groups (same as baseline)
    G_CH = 13
    while True:
        n_grp = math.ceil(n_blk / G_CH)
        ok = True
        pool_slots = np.full((n_cores, P, n_blk), PAD_SLOT, dtype=np.float32)
        pool_base = np.zeros((n_cores, n_grp), dtype=np.int64)
        for k in range(n_cores):
            b = batch[k * n_pc:(k + 1) * n_pc]
            for g in range(n_grp):
                lo = g * G_CH * P
                if lo >= n_pc:
                    pool_base[k, g] = 0
                    continue
                hi = min((g + 1) * G_CH * P, n_pc)
                base = b[lo]
                pool_base[k, g] = base
                rl = b[lo:hi] - base
                if rl.max() >= P:
                    ok = False
                    break
                sl = np.full(min((g + 1) * G_CH * P, n_blk * P) - lo, PAD_SLOT,
                             dtype=np.float32)
                sl[:hi - lo] = rl
                dstv = pool_slots[k].T.reshape(-1)
                dstv[lo:lo + len(sl)] = sl
                pool_slots[k] = dstv.reshape(n_blk, P).T
            if not ok:
                break
        if ok:
            break
        G_CH //= 2
        assert G_CH >= 1
    pl.G_CH, pl.n_grp = G_CH, n_grp
    pl.pool_slots, pl.pool_base = pool_slots, pool_base

    x = np.asarray(x, dtype=np.float32)
    x_sh = np.zeros((n_cores, n_pad, pl.d_in), dtype=BF16)
    x_sh[:, :n_pc] = x.reshape(n_cores, n_pc, pl.d_in).astype(BF16)
    pl.x_sh = x_sh
    pl.x_shT = np.ascontiguousarray(x_sh.transpose(0, 2, 1))  # [d_in, n_pad]

    W1 = np.asarray(W1, dtype=np.float32)
    W2 = np.asarray(W2, dtype=np.float32)
    kk = pl.d_in // P
    pl.w1t = np.concatenate([W1[k * P:(k + 1) * P] for k in range(kk)],
                            axis=1).astype(BF16)
    pl.n_k1 = kk
    pl.w2_sb = W2.astype(BF16)

    pl.b1b = np.broadcast_to(np.asarray(b1, np.float32), (P, pl.d_hid)).copy()
    pl.b2b = np.broadcast_to(np.asarray(b2, np.float32), (P, pl.d_out)).copy()
    iot = np.broadcast_to(np.arange(P, dtype=np.float32), (P, P))
    pl.iotab = iot.astype(BF16).copy()
    pl.iotab2 = (iot + 128.0).astype(BF16).copy()
    pl.iotaf = iot.astype(np.float32).copy()
    pl.ident = np.eye(P, dtype=BF16)
    return pl


def verify_plan(pl, x, W1, b1, W2, b2, edge_index, batch):
    """Numpy emulation of the device program, following exact instruction
    semantics (gather + one-hot matmuls), vs the reference math."""
    n_pc, n_blk, NWl = pl.n_pc, pl.n_blk, NW
    dis_t = pl.dis_t

    def emulate_layer(y_sh, y_full, transposed):
        # y_sh: [cores, n_pad, d] own shard (dis-scaled messages, bf16)
        # y_full: [n_nodes_rows, d] gathered table (window remap order)
        d = y_sh.shape[2]
        out = np.zeros((pl.n_cores, pl.n_pad, d), np.float32)
        for k in range(pl.n_cores):
            idx16 = pl.idx16_all[k]
            slots = np.asarray(pl.slots_all[k], np.float32)
            for m, (lo, hi) in enumerate(pl.groups):
                for wdw in range(NWl):
                    cols = int(pl.inst_cols[m, wdw])
                    if cols == 0:
                        continue
                    num = cols * P
                    ioff = int(pl.inst_idx_off[m, wdw])
                    unwrapped = idx16[:16, ioff:ioff + cols * 8] \
                        .T.reshape(-1)[:num]
                    gathered = y_full[wdw * pl.Wrows + unwrapped.astype(np.int64)]
                    gt = gathered.reshape(cols, P, d).transpose(1, 0, 2)
                    c0 = int(pl.col_w_start[m, wdw])
                    for b in range(lo, hi):
                        s = int(pl.s_bw[b, wdw])
                        if s == 0:
                            continue
                        t = int(pl.seg_t[b, wdw])
                        for lc in range(t // P, (t + s - 1) // P + 1):
                            sl = slots[:, c0 + lc]
                            onehot = (sl[:, None] ==
                                      (np.arange(P)[None, :]
                                       + 128 * (b % 2))).astype(np.float32)
                            g = gt[:, lc, :].astype(np.float32)
                            out[k, b * P:(b + 1) * P] += onehot.T @ g
            # self loop
            out[k] += np.asarray(y_sh[k], np.float32)
        return out

    # front-end
    x_f = np.asarray(pl.x_sh, np.float32)
    w1 = np.asarray(pl.w1t, np.float32)
    d_hid = pl.d_hid
    h1 = np.zeros((pl.n_cores, pl.n_pad, d_hid), np.float32)
    for k in range(pl.n_k1):
        h1 += x_f[:, :, k * P:(k + 1) * P] @ w1[:, k * d_hid:(k + 1) * d_hid]
    dis_flat = np.zeros((pl.n_cores, pl.n_pad), np.float32)
    for k in range(pl.n_cores):
        dis_flat[k] = pl.dis_t[k].T.reshape(-1)
    y1 = (h1 * dis_flat[:, :, None]).astype(BF16)

    def to_full(y_sh):
        full = np.zeros((pl.n_nodes, y_sh.shape[2]), y_sh.dtype)
        for j in range(NWl):
            for k in range(pl.n_cores):
                full[j * pl.Wrows + k * pl.Q:
                     j * pl.Wrows + (k + 1) * pl.Q] = \
                    y_sh[k, j * pl.Q:(j + 1) * pl.Q]
        return full

    y1_full = to_full(y1)
    agg1 = emulate_layer(y1, y1_full, False)
    z1 = np.maximum(agg1 * dis_flat[:, :, None] +
                    np.asarray(b1, np.float32)[None, None, :], 0.0)
    z1 = (z1 * dis_flat[:, :, None]).astype(BF16)
    z1_full = to_full(z1)
    agg2 = emulate_layer(z1, z1_full, True)
    h2 = agg2.astype(np.float32) @ np.asarray(pl.w2_sb, np.float32)
    z2 = h2 * dis_flat[:, :, None] + np.asarray(b2, np.float32)[None, None, :]
    recip_flat = np.zeros((pl.n_cores, pl.n_pad), np.float32)
    for k in range(pl.n_cores):
        recip_flat[k] = pl.recip_t[k].T.reshape(-1)
    z2 = z2 * recip_flat[:, :, None]

    # pool
    out = np.zeros((pl.num_graphs, pl.d_out), np.float32)
    batch = np.asarray(batch, np.int64)
    for k in range(pl.n_cores):
        bb = batch[k * n_pc:(k + 1) * n_pc]
        np.add.at(out, bb, z2[k, :n_pc])
    return out


def build_program(pl, body_repeat=1):
    import concourse.bass as bass
    import concourse.mybir as mybir
    import concourse.tile as tile
    from concourse import bacc
    from concourse import library_config

    f32 = mybir.dt.float32
    bf16 = mybir.dt.bfloat16
    i16 = mybir.dt.int16
    AF = mybir.ActivationFunctionType
    OP = mybir.AluOpType

    n_pc, n_blk, n_pad = pl.n_pc, pl.n_blk, pl.n_pad
    d_in, d_hid, d_out = pl.d_in, pl.d_hid, pl.d_out
    n_cores = pl.n_cores
    C_tot = pl.C_tot
    Q, Wrows = pl.Q, pl.Wrows

    nc = bacc.Bacc("TRN2", target_bir_lowering=False, debug=False,
                   num_devices=n_cores, num_swdge_queues=4)

    # --- I/O ---
    x_shT = nc.dram_tensor("x_shT", [d_in, n_pad], bf16, kind="ExternalInput")
    w1t_d = nc.dram_tensor("w1t", [P, pl.n_k1 * d_hid], bf16, kind="ExternalInput")
    w2_d = nc.dram_tensor("w2", [d_hid, d_out], bf16, kind="ExternalInput")
    b1b_d = nc.dram_tensor("b1b", [P, d_hid], f32, kind="ExternalInput")
    b2b_d = nc.dram_tensor("b2b", [P, d_out], f32, kind="ExternalInput")
    iotaf_d = nc.dram_tensor("iotaf", [P, P], f32, kind="ExternalInput")
    dis_d = nc.dram_tensor("dis_t", [P, n_blk], f32, kind="ExternalInput")
    recip_d = nc.dram_tensor("recip_t", [P, n_blk], f32, kind="ExternalInput")
    idx16_d = nc.dram_tensor("idx16", [P, C_tot * 8], i16, kind="ExternalInput")
    slots2_d = nc.dram_tensor("slots2", [P, 2 * C_tot], bf16,
                              kind="ExternalInput")
    iorep0_d = nc.dram_tensor("iorep0", [P, pl.max_span * P], bf16,
                              kind="ExternalInput")
    iorep1_d = nc.dram_tensor("iorep1", [P, pl.max_span * P], bf16,
                              kind="ExternalInput")
    pslots_d = nc.dram_tensor("pool_slots", [P, n_blk], f32, kind="ExternalInput")
    ident_d = nc.dram_tensor("ident", [P, P], bf16, kind="ExternalInput")

    pool_part = nc.dram_tensor("pool_part", [pl.n_grp * P, d_out], f32,
                               kind="ExternalOutput")

    # --- internal DRAM ---
    y1_sh = nc.dram_tensor("y1_sh", [n_pad, d_hid], bf16)
    z1_sh = nc.dram_tensor("z1_sh", [n_pad, d_hid], bf16)
    y1_full = nc.dram_tensor("y1_full", [pl.n_nodes, d_hid], bf16,
                             addr_space="Shared")
    y2_full = nc.dram_tensor("y2_full", [pl.n_nodes, d_hid], bf16,
                             addr_space="Shared")

    groups_rg = [list(range(n_cores))]

    with tile.TileContext(nc) as tc:
        with (
            tc.tile_pool(name="const", bufs=1) as cpool,
            tc.tile_pool(name="sb", bufs=5) as sb,
            tc.tile_pool(name="sb2", bufs=3) as sb2,
            tc.tile_pool(name="pgt", bufs=6) as pgt,
            tc.tile_pool(name="pmt", bufs=4) as pmt,
            tc.tile_pool(name="ps_agg", bufs=4, space="PSUM") as ps_agg,
            tc.tile_pool(name="ps_fe", bufs=2, space="PSUM") as ps_fe,
            tc.tile_pool(name="ps_o", bufs=1, space="PSUM") as ps_o,
            tc.tile_pool(name="ps_p", bufs=1, space="PSUM") as ps_p,
        ):
            w1_sb = cpool.tile([P, pl.n_k1 * d_hid], bf16)
            nc.sync.dma_start(out=w1_sb[:], in_=w1t_d[:, :])
            w2_sb = cpool.tile([d_hid, d_out], bf16)
            nc.sync.dma_start(out=w2_sb[:], in_=w2_d[:, :])
            b1_sb = cpool.tile([P, d_hid], f32)
            nc.sync.dma_start(out=b1_sb[:], in_=b1b_d[:, :])
            b2_sb = cpool.tile([P, d_out], f32)
            nc.sync.dma_start(out=b2_sb[:], in_=b2b_d[:, :])
            iof_sb = cpool.tile([P, P], f32)
            nc.sync.dma_start(out=iof_sb[:], in_=iotaf_d[:, :])
            dis_sb = cpool.tile([P, n_blk], f32)
            nc.sync.dma_start(out=dis_sb[:], in_=dis_d[:, :])
            recip_sb = cpool.tile([P, n_blk], f32)
            nc.sync.dma_start(out=recip_sb[:], in_=recip_d[:, :])
            idx_sb = cpool.tile([P, C_tot * 8], i16)
            nc.sync.dma_start(out=idx_sb[:], in_=idx16_d[:, :])
            slots2_sb = cpool.tile([P, 2 * C_tot], bf16)
            nc.sync.dma_start(out=slots2_sb[:], in_=slots2_d[:, :])
            iorep0_sb = cpool.tile([P, pl.max_span * P], bf16)
            iorep1_sb = cpool.tile([P, pl.max_span * P], bf16)
            iorep_sb = [iorep0_sb, iorep1_sb]
            nc.sync.dma_start(out=iorep_sb[0][:], in_=iorep0_d[:, :])
            nc.sync.dma_start(out=iorep_sb[1][:], in_=iorep1_d[:, :])
            pslots_sb = cpool.tile([P, n_blk], f32)
            nc.sync.dma_start(out=pslots_sb[:], in_=pslots_d[:, :])
            ident_sb = cpool.tile([P, P], bf16)
            nc.sync.dma_start(out=ident_sb[:], in_=ident_d[:, :])
            z2buf = cpool.tile([P, n_blk * d_out], f32)
            y1buf = cpool.tile([P, n_blk * d_hid], bf16)
            z1buf = cpool.tile([P, n_blk * d_hid], bf16)

            max_gt_cols = int(pl.inst_cols.max())
            for _w in range(6):
                gtw = pgt.tile([P, max_gt_cols * d_hid], bf16, tag="gt")
                nc.vector.memset(gtw[:], 0.0)

            for _rep in range(body_repeat):
                # ---------- front-end: y1 = dis * (x @ W1) ----------
                # wide loads of host-pre-transposed x: 8 node-blocks per DMA
                # (2KB/partition descriptors instead of per-block xbar
                # transposes at 256B/descriptor)
                FE_W = 8
                ag_bnd1 = {}
                for j in range(NW):
                    ag_bnd1[math.ceil((j + 1) * Q / P) - 1] = j
                for g0 in range(0, n_blk, FE_W):
                    nb = min(FE_W, n_blk - g0)
                    xw = {}
                    for k in range(pl.n_k1):
                        xwk = sb.tile([P, nb * P], bf16, tag="xw")
                        nc.sync.dma_start(
                            out=xwk[:],
                            in_=x_shT[k * P:(k + 1) * P,
                                      g0 * P:(g0 + nb) * P])
                        xw[k] = xwk
                    for j in range(nb):
                        g = g0 + j
                        psum_h = ps_fe.tile([P, d_hid], f32, tag="feps")
                        for k in range(pl.n_k1):
                            nc.tensor.matmul(
                                psum_h[:],
                                lhsT=xw[k][:, j * P:(j + 1) * P],
                                rhs=w1_sb[:, k * d_hid:(k + 1) * d_hid],
                                start=(k == 0), stop=(k == pl.n_k1 - 1))
                        yslice = y1buf[:, g * d_hid:(g + 1) * d_hid]
                        nc.scalar.activation(yslice, psum_h[:], AF.Copy,
                                             scale=dis_sb[:, g:g + 1])
                        nc.sync.dma_start(out=y1_sh[g * P:(g + 1) * P, :],
                                          in_=yslice)
                        if g in ag_bnd1:
                            jj = ag_bnd1[g]
                            nc.gpsimd.collective_compute(
                                "AllGather", OP.bypass,
                                replica_groups=groups_rg,
                                ins=[y1_sh[jj * Q:(jj + 1) * Q, :]],
                                outs=[y1_full[jj * Wrows:
                                              (jj + 1) * Wrows, :]])

                def do_layer(y_full, selfbuf, transposed):
                    """Aggregate messages; returns per-block epilogue hook."""
                    qrr = 0
                    SUB = 8  # max 8 cols = 1024 idx per dma_gather (fw cap)
                    for m, (lo, hi) in enumerate(pl.groups):
                        gts = {}
                        for wdw in range(NW):
                            cols = int(pl.inst_cols[m, wdw])
                            if cols == 0:
                                continue
                            ioff = int(pl.inst_idx_off[m, wdw])
                            gt = pgt.tile([P, cols * d_hid], bf16, tag="gt")
                            tot = int(pl.inst_tot[m, wdw])
                            for k0 in range(0, cols, SUB):
                                sc = min(SUB, cols - k0)
                                nidx = min(sc * P,
                                           (tot - k0 * P + 15) // 16 * 16)
                                nc.gpsimd.dma_gather(
                                    gt[:, k0 * d_hid:(k0 + sc) * d_hid]
                                        .rearrange("p (c e) -> p c e", e=d_hid),
                                    y_full[wdw * Wrows:(wdw + 1) * Wrows, :],
                                    idx_sb[:, ioff + k0 * 8:ioff + (k0 + sc) * 8],
                                    nidx, nidx, d_hid,
                                    single_packet=False,
                                    queue_num=qrr)
                                qrr = (qrr + 1) % 4
                            gts[wdw] = gt
                        for b in range(lo, hi):
                            sl = selfbuf[:, b * d_hid:(b + 1) * d_hid]
                            # batched one-hot build: one tensor_tensor per
                            # (block, window) span; parity-encoded slots
                            # disambiguate straddle columns (odd blocks +128)
                            iore = iorep_sb[b % 2]
                            chunks = []
                            mtbs = {}
                            for wdw in range(NW):
                                s = int(pl.s_bw[b, wdw])
                                if s == 0:
                                    continue
                                lc0 = int(pl.span_lo[b, wdw])
                                ns = int(pl.span_n[b, wdw])
                                for lc in range(lc0, lc0 + ns):
                                    chunks.append((wdw, lc))
                                mtb = pmt.tile([P, ns * P], bf16, tag="mtb")
                                c0 = int(pl.col_w_start[m, wdw]) + lc0
                                in0 = (slots2_sb[:, 2 * c0:2 * (c0 + ns)]
                                       .rearrange("p (c e) -> p c e", e=2)
                                       .unsqueeze(2)
                                       .broadcast_to([P, ns, 64, 2]))
                                nc.vector.tensor_tensor(
                                    out=mtb[:], in0=in0,
                                    in1=iore[:, :ns * P], op=OP.is_equal)
                                mtbs[wdw] = (mtb, lc0)
                            psz = P if transposed else d_hid
                            psum_a = ps_agg.tile([P, psz], f32, tag="agg")
                            seed_stop = len(chunks) == 0
                            if transposed:
                                nc.tensor.matmul(psum_a[:], lhsT=sl,
                                                 rhs=ident_sb[:],
                                                 start=True, stop=seed_stop)
                            else:
                                nc.tensor.matmul(psum_a[:], lhsT=ident_sb[:],
                                                 rhs=sl,
                                                 start=True, stop=seed_stop)
                            for ci, (wdw, lc) in enumerate(chunks):
                                last = ci == len(chunks) - 1
                                mtb, lc0 = mtbs[wdw]
                                mtc = mtb[:, (lc - lc0) * P:(lc - lc0 + 1) * P]
                                gtc = gts[wdw][:, lc * d_hid:(lc + 1) * d_hid]
                                if transposed:
                                    nc.tensor.matmul(psum_a[:], lhsT=gtc,
                                                     rhs=mtc,
                                                     start=False, stop=last)
                                else:
                                    nc.tensor.matmul(psum_a[:], lhsT=mtc,
                                                     rhs=gtc,
                                                     start=False, stop=last)
                            yield b, psum_a

                # ---------- layer 1 ----------
                # AG2 chunk j covers z1_sh rows [j*Q,(j+1)*Q): issue it as
                # soon as the last block covering those rows is done so the
                # collective overlaps the rest of layer-1 aggregation
                ag_bnd = {}
                for j in range(NW):
                    ag_bnd[math.ceil((j + 1) * Q / P) - 1] = j
                for b, psum_a in do_layer(y1_full, y1buf, False):
                    t1 = sb2.tile([P, d_hid], f32, tag="ep1")
                    nc.scalar.activation(t1[:], psum_a[:], AF.Copy,
                                         scale=dis_sb[:, b:b + 1])
                    t2 = sb2.tile([P, d_hid], f32, tag="ep2")
                    nc.vector.tensor_tensor(t2[:], t1[:], b1_sb[:], op=OP.add)
                    zslice = z1buf[:, b * d_hid:(b + 1) * d_hid]
                    nc.vector.tensor_scalar(out=zslice, in0=t2[:],
                                            scalar1=0.0,
                                            scalar2=dis_sb[:, b:b + 1],
                                            op0=OP.max, op1=OP.mult)
                    nc.sync.dma_start(out=z1_sh[b * P:(b + 1) * P, :],
                                      in_=zslice)
                    if b in ag_bnd:
                        j = ag_bnd[b]
                        nc.gpsimd.collective_compute(
                            "AllGather", OP.bypass, replica_groups=groups_rg,
                            ins=[z1_sh[j * Q:(j + 1) * Q, :]],
                            outs=[y2_full[j * Wrows:(j + 1) * Wrows, :]])

                # ---------- layer 2 (transposed accumulation) ----------
                for b, psum_t in do_layer(y2_full, z1buf, True):
                    s2t = sb2.tile([P, P], bf16, tag="s2t")
                    nc.scalar.activation(s2t[:], psum_t[:], AF.Copy)
                    psum_o = ps_o.tile([P, d_out], f32, tag="out2")
                    nc.tensor.matmul(psum_o[:], lhsT=s2t[:], rhs=w2_sb[:],
                                     start=True, stop=True)
                    t3 = sb2.tile([P, d_out], f32, tag="ep3")
                    nc.scalar.activation(t3[:], psum_o[:], AF.Copy,
                                         scale=dis_sb[:, b:b + 1])
                    t4 = sb2.tile([P, d_out], f32, tag="ep4")
                    nc.vector.tensor_tensor(t4[:], t3[:], b2_sb[:], op=OP.add)
                    nc.vector.tensor_scalar(
                        out=z2buf[:, b * d_out:(b + 1) * d_out],
                        in0=t4[:], scalar1=recip_sb[:, b:b + 1], scalar2=None,
                        op0=OP.mult)

                # ---------- pool ----------
                for grp in range(pl.n_grp):
                    lo = grp * pl.G_CH
                    hi = min((grp + 1) * pl.G_CH, n_blk)
                    psum_p = ps_p.tile([P, d_out], f32, tag="pool")
                    for j, cblk in enumerate(range(lo, hi)):
                        mp = sb.tile([P, P], f32, tag="poolM")
                        nc.vector.tensor_tensor(
                            out=mp[:],
                            in0=pslots_sb[:, cblk:cblk + 1].to_broadcast([P, P]),
                            in1=iof_sb[:], op=OP.is_equal)
                        nc.tensor.matmul(
                            psum_p[:], lhsT=mp[:],
                            rhs=z2buf[:, cblk * d_out:(cblk + 1) * d_out],
                            start=(j == 0), stop=(j == hi - lo - 1))
                    pout = sb.tile([P, d_out], f32, tag="pout")
                    nc.vector.tensor_copy(out=pout[:], in_=psum_p[:])
                    nc.sync.dma_start(out=pool_part[grp * P:(grp + 1) * P, :],
                                      in_=pout[:])

    nc.compile()
    return nc


def make_in_maps(pl):
    maps = []
    for k in range(pl.n_cores):
        maps.append({
            "x_shT": pl.x_shT[k],
            "w1t": pl.w1t,
            "w2": pl.w2_sb,
            "b1b": pl.b1b,
            "b2b": pl.b2b,
            "iotaf": pl.iotaf,
            "dis_t": pl.dis_t[k],
            "recip_t": pl.recip_t[k],
            "idx16": pl.idx16_all[k],
            "slots2": pl.slots2[k],
            "iorep0": pl.iorep0,
            "iorep1": pl.iorep1,
            "pool_slots": pl.pool_slots[k],
            "ident": pl.ident,
        })
    return maps


def combine_outputs(pl, parts):
    out = np.zeros((pl.num_graphs, pl.d_out), dtype=np.float32)
    for k in range(pl.n_cores):
        pp = np.asarray(parts[k], dtype=np.float32).reshape(pl.n_grp, P, pl.d_out)
        for g in range(pl.n_grp):
            base = int(pl.pool_base[k, g])
            n = min(P, pl.num_graphs - base)
            if n > 0:
                out[base:base + n] += pp[g, :n]
    return out


def make_pjrt_runner(nc, in_maps, n_cores):
    import jax
    import numpy as np
    from jax.sharding import Mesh, PartitionSpec
    from jax.experimental.shard_map import shard_map
    import concourse.mybir as mybir
    from concourse.bass2jax import (
        _bass_exec_p, install_neuronx_cc_hook, partition_id_tensor)

    install_neuronx_cc_hook()
    assert nc.dbg_addr is None or not nc.dbg_callbacks

    partition_name = nc.partition_id_tensor.name if nc.partition_id_tensor else None
    in_names, out_names, out_avals, zero_outs = [], [], [], []
    for alloc in nc.m.functions[0].allocations:
        if not isinstance(alloc, mybir.MemoryLocationSet):
            continue
        name = alloc.memorylocations[0].name
        if alloc.kind == "ExternalInput":
            if name != partition_name:
                in_names.append(name)
        elif alloc.kind == "ExternalOutput":
            shape = tuple(alloc.tensor_shape)
            dtype = mybir.dt.np(alloc.dtype)
            out_names.append(name)
            out_avals.append(jax.core.ShapedArray(shape, dtype))
            zero_outs.append(np.zeros(shape, dtype))
    n_params = len(in_names)
    all_names = list(in_names) + list(out_names)
    if partition_name is not None:
        all_names.append(partition_name)

    def _body(*args):
        operands = list(args)
        if partition_name is not None:
            operands.append(partition_id_tensor())
        outs = _bass_exec_p.bind(
            *operands,
            out_avals=tuple(out_avals),
            in_names=tuple(all_names),
            out_names=tuple(out_names),
            lowering_input_output_aliases=(),
            sim_require_finite=True,
            sim_require_nnan=True,
            nc=nc,
        )
        return tuple(outs)

    devices = jax.devices()[:n_cores]
    mesh = Mesh(np.asarray(devices), ("core",))
    n_outs = len(out_names)
    in_specs = (PartitionSpec("core"),) * (n_params + n_outs)
    out_specs = (PartitionSpec("core"),) * n_outs
    fn = jax.jit(shard_map(_body, mesh=mesh, in_specs=in_specs,
                           out_specs=out_specs, check_rep=False),
                 keep_unused=True)
    per_core = [[np.asarray(m[name]) for name in in_names] for m in in_maps]
    concat_in = [np.concatenate([per_core[c][i] for c in range(n_cores)], axis=0)
                 for i in range(n_params)]
    concat_zeros = [np.zeros((n_cores * z.shape[0], *z.shape[1:]), z.dtype)
                    for z in zero_outs]
    args = concat_in + concat_zeros
    out_shapes = [a.shape for a in out_avals]
    return fn, args, out_names, out_shapes


def prepare(x, W1, b1, W2, b2, edge_index, batch):
    pl = make_plan(x, W1, b1, W2, b2, edge_index, batch)
    nc = build_program(pl)
    in_maps = make_in_maps(pl)
    return pl, nc, in_maps


def kernel(x, W1, b1, W2, b2, edge_index, batch):
    from concourse.bass_utils import run_bass_kernel_spmd

    pl, nc, in_maps = prepare(x, W1, b1, W2, b2, edge_index, batch)
    res = run_bass_kernel_spmd(nc, in_maps, list(range(pl.n_cores)))
    parts = [res.results[k]["pool_part"] for k in range(pl.n_cores)]
    return combine_outputs(pl, parts)

